# revision 23
# baseline (speedup 1.0000x reference)
"""ConvDualAttention Trainium2 kernel (Bass/Tile), 8-core data-parallel.

Contract: kernel(**inputs) takes the FULL unsharded inputs, shards batch b
across the 8 NeuronCores (one batch per core), and returns the full
(8, 128, 4096) float32 output.

Math (per batch b, per head h, D=128, X=4096):
  y_p   = dwconv3(x) + t_p/s_p           (p in q,k,v; BN folded so that
                                          W_eff_p @ y_p == pw_p @ BN(conv))
  k     = W_eff_k @ y_k ; sk = softmax(k over d)
  kat   = SCALE * q^T @ sk               (SCALE folded into W_q)
  gout  = GW @ q + gb ; sig = sigmoid(gout)
  out_h = v @ kat + sig^T * v
  out   = out_w @ merge(out_h) + out_b

Kernel factorizations (validated against the jax reference):
  * q is never materialized:  kat_h = wtq_h^T @ R_h with
    R_h = y_qT^T @ sk_h (contraction over x).  y_qT (the transposed,
    bias-included conv output) is produced by PE transposes of y_q.
  * v@kat through the output projection collapses to W3 @ y_v with
    W3 = sum_h outw_h @ (Wv_h^T @ kat_h)^T, computed on-chip.

Schedule: gate-path chunks (gout/v/sigmoid/gate-mult) are interleaved into
the softmax-normalize windows where Act/PE would otherwise idle; the kat/W3
chain and final projections run last with their own PSUM pools.
"""
import numpy as np
import ml_dtypes

import concourse.bass as bass
import concourse.tile as tile
from concourse import bacc, mybir
from concourse.bass_utils import run_bass_kernel_spmd
from concourse.masks import make_identity

F32 = mybir.dt.float32
F32R = mybir.dt.float32r
BF16 = mybir.dt.bfloat16
AF = mybir.ActivationFunctionType
ALU = mybir.AluOpType

B = 8
DIM = 128
HEADS = 8
INNER = DIM * HEADS
X = 4096
EPS = 1e-5
SCALE = DIM ** -0.5
NT = X // 128          # 32 x-tiles of 128
NCH = X // 512         # 8 chunks of 512
GROUPS = 2
GH = HEADS // GROUPS   # 4 heads per group

_NC = None
TRACE = False
LAST_EXEC_NS = None


def _bf(a):
    return np.ascontiguousarray(np.asarray(a, np.float32).astype(ml_dtypes.bfloat16))


def _prep(inputs):
    """Host-side weight folding. Returns dict of DRAM input arrays."""
    f = lambda k: np.asarray(inputs[k], np.float32)
    wt = {}
    tprime = {}
    diag_cols = []
    for p in ("q", "k", "v"):
        s = f(p + "_g") / np.sqrt(f(p + "_v") + EPS)        # (128,)
        t = f(p + "_b") - f(p + "_m") * s
        tprime[p] = t / s
        w_eff = f(p + "_pw") * s[None, :]                    # (1024, 128)
        wt[p] = np.ascontiguousarray(w_eff.T)                # (128, 1024)
        dw = f(p + "_dw")[:, 0, :]                           # (128, 3)
        for j in range(3):
            diag_cols.append(np.diag(dw[:, j]).astype(np.float32))
    s_gt = f("gt_g") / np.sqrt(f("gt_v") + EPS)
    t_gt = f("gt_b") - f("gt_m") * s_gt
    gw = f("gt_pw") * (f("gt_dw")[:, 0, 0] * s_gt)[None, :]  # (128, 128)
    gb = f("gt_pw") @ t_gt                                   # (128,)
    w_eff_q = wt["q"].T                                      # (1024, 128)
    gqt = np.concatenate(
        [(gw @ w_eff_q[h * 128:(h + 1) * 128, :]).T for h in range(HEADS)], axis=1
    )                                                        # (128 i, 1024 h*o)
    out_w = f("out_w")                                       # (128, 1024)
    outwt = np.concatenate(
        [np.ascontiguousarray(out_w[:, h * 128:(h + 1) * 128].T) for h in range(HEADS)],
        axis=1,
    )                                                        # (128 d, 1024 h*o)
    wvdm = np.concatenate(
        [wt["v"].T[h * 128:(h + 1) * 128, :] for h in range(HEADS)], axis=1
    )                                                        # (128 d, 1024 h*i)
    diag = np.concatenate(diag_cols, axis=1)                 # (128, 1152)
    wtq_s = wt["q"] * SCALE                                  # (128 i, 1024 d)
    biasp = np.stack(
        [tprime["q"], tprime["k"], tprime["v"], gb, f("out_b")], axis=1
    )                                                        # (128, 5)
    return {
        "wtk": _bf(wt["k"]),
        "wtv": _bf(wt["v"]),
        "gqt": _bf(gqt),
        "outwt": _bf(outwt),
        "wvdm": _bf(wvdm),
        "diag": _bf(diag),
        "biasp": np.ascontiguousarray(biasp),
        "wtqr": _bf(wtq_s),
    }


def _build():
    nc = bacc.Bacc("TRN2", target_bir_lowering=False, debug=False, num_devices=B)
    xb_d = nc.dram_tensor("xb", [128, X + 2], BF16, kind="ExternalInput").ap()
    wtk_d = nc.dram_tensor("wtk", [128, INNER], BF16, kind="ExternalInput").ap()
    wtv_d = nc.dram_tensor("wtv", [128, INNER], BF16, kind="ExternalInput").ap()
    gqt_d = nc.dram_tensor("gqt", [128, INNER], BF16, kind="ExternalInput").ap()
    outwt_d = nc.dram_tensor("outwt", [128, INNER], BF16, kind="ExternalInput").ap()
    wvdm_d = nc.dram_tensor("wvdm", [128, INNER], BF16, kind="ExternalInput").ap()
    diag_d = nc.dram_tensor("diag", [128, 9 * 128], BF16, kind="ExternalInput").ap()
    biasp_d = nc.dram_tensor("biasp", [128, 5], F32, kind="ExternalInput").ap()
    wtqr_d = nc.dram_tensor("wtqr", [128, INNER], BF16, kind="ExternalInput").ap()
    out_d = nc.dram_tensor("out", [128, X], F32, kind="ExternalOutput").ap()

    with tile.TileContext(nc) as tc:
        with (
            tc.tile_pool(name="const", bufs=1) as cp,
            tc.tile_pool(name="gp2", bufs=1) as gp2,
            tc.tile_pool(name="gatep", bufs=1) as gatep,
            tc.tile_pool(name="small", bufs=2) as sp,
        ):
            wtk = cp.tile([128, INNER], BF16)
            wtv = cp.tile([128, INNER], BF16)
            gqt = cp.tile([128, INNER], BF16)
            outwt = cp.tile([128, INNER], BF16)
            wvdm = cp.tile([128, INNER], BF16)
            biasp = cp.tile([128, 5], F32)
            wtqr = cp.tile([128, INNER], BF16)
            ident = cp.tile([128, 128], BF16)
            yq = cp.tile([128, X], BF16, tag="yq")
            yv = cp.tile([128, X], BF16, tag="yv")
            yqt = cp.tile([128, X], BF16, tag="yqt")
            zt = cp.tile([128, 2 * NT * GH], BF16, tag="zt")
            zs = cp.tile([128, 2 * NT], F32, tag="zs")
            z4a = cp.tile([128, 8], F32, tag="z4a")
            z4b = cp.tile([128, 8], F32, tag="z4b")
            zi = cp.tile([128, 2 * NT * GH], F32, tag="zi")
            w3t_sb = cp.tile([128, 128], BF16, tag="w3t")
            sks = [gp2.tile([128, NT * 512], BF16, tag=f"sksb{g}",
                            name=f"sksb{g}") for g in range(GROUPS)]
            gates = [gatep.tile([128, HEADS * 512], BF16, tag=f"gate{c}",
                                name=f"gate{c}") for c in range(NCH)]
            r_sbs = [sp.tile([128, 512], BF16, tag=f"rsb{g}",
                             name=f"rsb{g}") for g in range(GROUPS)]

            make_identity(nc, ident)

            ys = {"q": yq, "v": yv}

            # ---- region 1: conv (q,k,v) + K logits + exp + Z ----
            with (
                tc.tile_pool(name="xp", bufs=1) as xp,
                tc.tile_pool(name="yps", bufs=2, space="PSUM") as yps,
                tc.tile_pool(name="tps", bufs=2, space="PSUM") as tps,
                tc.tile_pool(name="kqps", bufs=3, space="PSUM") as kqps,
            ):
                diag = xp.tile([128, 9 * 128], BF16)
                yk = xp.tile([128, X], BF16, tag="yk")
                xpc = [xp.tile([128, 514], BF16, tag=f"xp{c}", name=f"xpc{c}")
                       for c in range(NCH)]
                nc.sync.dma_start(out=diag, in_=diag_d)
                nc.sync.dma_start(out=biasp, in_=biasp_d)
                for c in range(NCH):
                    nc.sync.dma_start(
                        out=xpc[c], in_=xb_d[:, c * 512:c * 512 + 514])
                for sb_t, dr in ((wtk, wtk_d), (wtqr, wtqr_d), (wvdm, wvdm_d),
                                 (outwt, outwt_d), (gqt, gqt_d), (wtv, wtv_d)):
                    nc.sync.dma_start(out=sb_t, in_=dr)

                for pi, p in enumerate(("q", "k", "v")):
                    for c in range(NCH):
                        pt = yps.tile([128, 512], F32, tag="yps")
                        for j in range(3):
                            dsl = diag[:, (pi * 3 + j) * 128:(pi * 3 + j + 1) * 128]
                            nc.tensor.matmul(
                                pt, dsl, xpc[c][:, j:j + 512],
                                start=(j == 0), stop=(j == 2),
                            )
                        out_t = yk if p == "k" else ys[p]
                        nc.scalar.activation(
                            out_t[:, c * 512:(c + 1) * 512], pt,
                            AF.Identity, bias=biasp[:, pi:pi + 1],
                        )
                        if p == "q":
                            for tt in range(4):
                                t = c * 4 + tt
                                qt = tps.tile([128, 128], BF16, tag="yqt")
                                nc.tensor.transpose(
                                    qt, yq[:, t * 128:(t + 1) * 128], ident,
                                )
                                nc.vector.tensor_copy(
                                    yqt[:, t * 128:(t + 1) * 128], qt
                                )
                def normalize(g, act_mod):
                    # normalize sk in place; every act_mod-th slice on Act
                    sksb = sks[g]
                    for t in range(NT):
                        for hh in range(GH):
                            col = g * NT * GH + t * GH + hh
                            sl = sksb[:, t * 512 + hh * 128:
                                      t * 512 + (hh + 1) * 128]
                            if (t * GH + hh) % 16 < act_mod:
                                nc.scalar.activation(
                                    sl, sl, AF.Copy, scale=zi[:, col:col + 1]
                                )
                            else:
                                nc.vector.tensor_scalar(
                                    sl, sl, zi[:, col:col + 1], None, ALU.mult
                                )

                for g in range(GROUPS):
                    osl = slice(g * 512, (g + 1) * 512)
                    for t in range(NT):
                        kps = kqps.tile([128, 512], F32, tag="kq")
                        nc.tensor.matmul(
                            kps, yk[:, t * 128:(t + 1) * 128], wtk[:, osl],
                            start=True, stop=True,
                        )
                        nc.scalar.activation(
                            sks[g][:, t * 512:(t + 1) * 512], kps, AF.Exp,
                        )
                        if t % 4 == 3:
                            t0 = t - 3
                            inr = sks[g][:, t0 * 512:(t + 1) * 512].rearrange(
                                "p (t h d) -> p t h d", t=4, h=GH
                            )
                            zb4 = g * NT * GH + t0 * GH
                            with nc.allow_low_precision("bf16 softmax denom"):
                                nc.vector.tensor_reduce(
                                    zt[:, zb4:zb4 + 4 * GH], inr,
                                    mybir.AxisListType.X, ALU.add,
                                )
                        if t % 8 == 7:
                            zb8 = g * NT * GH + (t - 7) * GH
                            nc.vector.reciprocal(
                                zi[:, zb8:zb8 + 8 * GH], zt[:, zb8:zb8 + 8 * GH]
                            )
                    if g == 0:
                        # g0 normalize rides under g1's EXP window (all-DVE)
                        normalize(0, 0)

            # ---- region 2: normalize + R, overlapped with gate-path ----
            with (
                tc.tile_pool(name="sigp", bufs=4) as sigp,
                tc.tile_pool(name="rps", bufs=1, space="PSUM") as rps,
                tc.tile_pool(name="goutps", bufs=2, space="PSUM") as goutps,
                tc.tile_pool(name="vps", bufs=4, space="PSUM") as vps,
            ):
                def gate_chunk(c):
                    # deep v_ps/sig rings so PE can run ahead of the DVE
                    # gate-mult drain while DVE is busy normalizing
                    csl = slice(c * 512, (c + 1) * 512)
                    for h in range(HEADS):
                        g_ps = goutps.tile([128, 512], F32, tag="gout")
                        nc.tensor.matmul(
                            g_ps, gqt[:, h * 128:(h + 1) * 128],
                            yq[:, csl], start=True, stop=True,
                        )
                        sig = sigp.tile([128, 512], BF16, tag="sig")
                        nc.scalar.activation(
                            sig, g_ps, AF.Sigmoid, bias=biasp[:, 3:4],
                        )
                        v_ps = vps.tile([128, 512], F32, tag="vp")
                        nc.tensor.matmul(
                            v_ps, wtv[:, h * 128:(h + 1) * 128],
                            yv[:, csl], start=True, stop=True,
                        )
                        nc.vector.tensor_tensor(
                            gates[c][:, h * 512:(h + 1) * 512], v_ps, sig,
                            ALU.mult,
                        )

                def r_group(g):
                    r_ps = rps.tile([128, 512], F32, tag="r")
                    for t in range(NT):
                        nc.tensor.matmul(
                            r_ps, yqt[:, t * 128:(t + 1) * 128],
                            sks[g][:, t * 512:(t + 1) * 512],
                            start=(t == 0), stop=(t == NT - 1),
                            skip_group_check=True,
                        )
                    nc.vector.tensor_copy(r_sbs[g], r_ps)

                # g0 already normalized during region 1
                gate_chunk(0)
                r_group(0)
                normalize(1, 4)
                gate_chunk(1)
                gate_chunk(2)
                gate_chunk(3)
                r_group(1)

            # ---- region 3a: gate chunks overlapped with kat/W3 chain ----
            with (
                tc.tile_pool(name="katps", bufs=1, space="PSUM") as katps,
                tc.tile_pool(name="m2ps", bufs=1, space="PSUM") as m2ps,
                tc.tile_pool(name="w3ps", bufs=1, space="PSUM") as w3ps,
                tc.tile_pool(name="sigp2", bufs=2) as sigp,
                tc.tile_pool(name="goutps2", bufs=2, space="PSUM") as goutps,
                tc.tile_pool(name="vps2", bufs=1, space="PSUM") as vps,
            ):
                def gate_chunk3(c):
                    csl = slice(c * 512, (c + 1) * 512)
                    for hp in range(HEADS // 2):
                        v_ps = vps.tile([128, 1024], F32, tag="vp")
                        sig = sigp.tile([128, 1024], BF16, tag="sig")
                        for d in range(2):
                            h = hp * 2 + d
                            g_ps = goutps.tile([128, 512], F32, tag="gout")
                            nc.tensor.matmul(
                                g_ps, gqt[:, h * 128:(h + 1) * 128],
                                yq[:, csl], start=True, stop=True,
                            )
                            nc.scalar.activation(
                                sig[:, d * 512:(d + 1) * 512], g_ps,
                                AF.Sigmoid, bias=biasp[:, 3:4],
                            )
                            nc.tensor.matmul(
                                v_ps[:, d * 512:(d + 1) * 512],
                                wtv[:, h * 128:(h + 1) * 128],
                                yv[:, csl], start=True, stop=True,
                            )
                        nc.vector.tensor_tensor(
                            gates[c][:, hp * 1024:(hp + 1) * 1024], v_ps, sig,
                            ALU.mult,
                        )

                def kat_chain(g):
                    for hh in range(GH):
                        h = g * GH + hh
                        kat_ps = katps.tile([128, 128], F32, tag="katp")
                        nc.tensor.matmul(
                            kat_ps, wtqr[:, h * 128:(h + 1) * 128],
                            r_sbs[g][:, hh * 128:(hh + 1) * 128],
                            start=True, stop=True, skip_group_check=True,
                        )
                        kat_sb = sp.tile([128, 128], BF16, tag="katsb")
                        nc.vector.tensor_copy(kat_sb, kat_ps)
                        m2_ps = m2ps.tile([128, 128], F32, tag="m2")
                        nc.tensor.matmul(
                            m2_ps, kat_sb,
                            wvdm[:, h * 128:(h + 1) * 128],
                            start=True, stop=True, skip_group_check=True,
                        )
                        m2_sb = sp.tile([128, 128], BF16, tag="m2sb")
                        nc.vector.tensor_copy(m2_sb, m2_ps)
                        nc.tensor.matmul(
                            w3t_ps, m2_sb,
                            outwt[:, h * 128:(h + 1) * 128],
                            start=(h == 0), stop=(h == HEADS - 1),
                            skip_group_check=True,
                        )

                w3t_ps = w3ps.tile([128, 128], F32)
                gate_chunk3(4)
                kat_chain(0)
                gate_chunk3(5)
                kat_chain(1)
                gate_chunk3(6)
                nc.vector.tensor_copy(w3t_sb, w3t_ps)

            # ---- region 3b: final projection per 512-chunk ----
            with (
                tc.tile_pool(name="finps", bufs=2, space="PSUM") as finps,
                tc.tile_pool(name="sigp3", bufs=2) as sigp,
                tc.tile_pool(name="goutps3", bufs=2, space="PSUM") as goutps,
                tc.tile_pool(name="vps3", bufs=1, space="PSUM") as vps,
                tc.tile_pool(name="bpool", bufs=2) as bp,
            ):

                def gate_chunk3(c):
                    csl = slice(c * 512, (c + 1) * 512)
                    for hp in range(HEADS // 2):
                        v_ps = vps.tile([128, 1024], F32, tag="vp")
                        sig = sigp.tile([128, 1024], BF16, tag="sig")
                        for d in range(2):
                            h = hp * 2 + d
                            g_ps = goutps.tile([128, 512], F32, tag="gout")
                            nc.tensor.matmul(
                                g_ps, gqt[:, h * 128:(h + 1) * 128],
                                yq[:, csl], start=True, stop=True,
                            )
                            nc.scalar.activation(
                                sig[:, d * 512:(d + 1) * 512], g_ps,
                                AF.Sigmoid, bias=biasp[:, 3:4],
                            )
                            nc.tensor.matmul(
                                v_ps[:, d * 512:(d + 1) * 512],
                                wtv[:, h * 128:(h + 1) * 128],
                                yv[:, csl], start=True, stop=True,
                            )
                        nc.vector.tensor_tensor(
                            gates[c][:, hp * 1024:(hp + 1) * 1024], v_ps, sig,
                            ALU.mult,
                        )

                def fin_chunk(c):
                    csl = slice(c * 512, (c + 1) * 512)
                    fin_ps = finps.tile([128, 512], F32, tag="fin")
                    nc.tensor.matmul(
                        fin_ps, w3t_sb, yv[:, csl],
                        start=True, stop=False, skip_group_check=True,
                    )
                    for h in range(HEADS):
                        nc.tensor.matmul(
                            fin_ps, outwt[:, h * 128:(h + 1) * 128],
                            gates[c][:, h * 512:(h + 1) * 512],
                            start=False, stop=(h == HEADS - 1),
                            skip_group_check=True,
                        )
                    fin_sb = bp.tile([128, 512], F32, tag="finsb")
                    nc.scalar.activation(
                        fin_sb, fin_ps, AF.Identity, bias=biasp[:, 4:5]
                    )
                    nc.sync.dma_start(out=out_d[:, csl], in_=fin_sb)

                gate_chunk3(7)
                for c in range(NCH):
                    fin_chunk(c)

    nc.compile()
    return nc


def kernel(**inputs):
    global _NC, LAST_EXEC_NS
    host = _prep(inputs)
    if _NC is None:
        _NC = _build()
    x = np.asarray(inputs["x"], np.float32)
    in_maps = []
    for b in range(B):
        xp = np.pad(x[b], ((0, 0), (1, 1)))
        m = {"xb": _bf(xp)}
        m.update(host)
        in_maps.append(m)
    res = run_bass_kernel_spmd(
        _NC, in_maps, core_ids=list(range(B)), trace=TRACE
    )
    LAST_EXEC_NS = res.exec_time_ns
    return np.stack([r["out"] for r in res.results]).astype(np.float32)


# revision 25
# speedup vs baseline: 1.0190x; 1.0190x over previous
"""ConvDualAttention Trainium2 kernel (Bass/Tile), 8-core data-parallel.

Contract: kernel(**inputs) takes the FULL unsharded inputs, shards batch b
across the 8 NeuronCores (one batch per core), and returns the full
(8, 128, 4096) float32 output.

Math (per batch b, per head h, D=128, X=4096):
  y_p   = dwconv3(x) + t_p/s_p           (p in q,k,v; BN folded so that
                                          W_eff_p @ y_p == pw_p @ BN(conv))
  k     = W_eff_k @ y_k ; sk = softmax(k over d)
  kat   = SCALE * q^T @ sk               (SCALE folded into W_q)
  gout  = GW @ q + gb ; sig = sigmoid(gout)
  out_h = v @ kat + sig^T * v
  out   = out_w @ merge(out_h) + out_b

Kernel factorizations (validated against the jax reference):
  * q is never materialized:  kat_h = wtq_h^T @ R_h with
    R_h = y_qT^T @ sk_h (contraction over x).  y_qT (the transposed,
    bias-included conv output) is produced by PE transposes of y_q.
  * v@kat through the output projection collapses to W3 @ y_v with
    W3 = sum_h outw_h @ (Wv_h^T @ kat_h)^T, computed on-chip.

Schedule: gate-path chunks (gout/v/sigmoid/gate-mult) are interleaved into
the softmax-normalize windows where Act/PE would otherwise idle; the kat/W3
chain and final projections run last with their own PSUM pools.
"""
import numpy as np
import ml_dtypes

import concourse.bass as bass
import concourse.tile as tile
from concourse import bacc, mybir
from concourse.bass_utils import run_bass_kernel_spmd
from concourse.masks import make_identity

F32 = mybir.dt.float32
F32R = mybir.dt.float32r
BF16 = mybir.dt.bfloat16
AF = mybir.ActivationFunctionType
ALU = mybir.AluOpType

B = 8
DIM = 128
HEADS = 8
INNER = DIM * HEADS
X = 4096
EPS = 1e-5
SCALE = DIM ** -0.5
NT = X // 128          # 32 x-tiles of 128
NCH = X // 512         # 8 chunks of 512
GROUPS = 2
GH = HEADS // GROUPS   # 4 heads per group

_NC = None
TRACE = False
LAST_EXEC_NS = None


def _bf(a):
    return np.ascontiguousarray(np.asarray(a, np.float32).astype(ml_dtypes.bfloat16))


def _prep(inputs):
    """Host-side weight folding. Returns dict of DRAM input arrays."""
    f = lambda k: np.asarray(inputs[k], np.float32)
    wt = {}
    tprime = {}
    diag_cols = []
    for p in ("q", "k", "v"):
        s = f(p + "_g") / np.sqrt(f(p + "_v") + EPS)        # (128,)
        t = f(p + "_b") - f(p + "_m") * s
        tprime[p] = t / s
        w_eff = f(p + "_pw") * s[None, :]                    # (1024, 128)
        wt[p] = np.ascontiguousarray(w_eff.T)                # (128, 1024)
        dw = f(p + "_dw")[:, 0, :]                           # (128, 3)
        for j in range(3):
            diag_cols.append(np.diag(dw[:, j]).astype(np.float32))
    s_gt = f("gt_g") / np.sqrt(f("gt_v") + EPS)
    t_gt = f("gt_b") - f("gt_m") * s_gt
    gw = f("gt_pw") * (f("gt_dw")[:, 0, 0] * s_gt)[None, :]  # (128, 128)
    gb = f("gt_pw") @ t_gt                                   # (128,)
    w_eff_q = wt["q"].T                                      # (1024, 128)
    gqt = np.concatenate(
        [(gw @ w_eff_q[h * 128:(h + 1) * 128, :]).T for h in range(HEADS)], axis=1
    )                                                        # (128 i, 1024 h*o)
    out_w = f("out_w")                                       # (128, 1024)
    outwt = np.concatenate(
        [np.ascontiguousarray(out_w[:, h * 128:(h + 1) * 128].T) for h in range(HEADS)],
        axis=1,
    )                                                        # (128 d, 1024 h*o)
    wvdm = np.concatenate(
        [wt["v"].T[h * 128:(h + 1) * 128, :] for h in range(HEADS)], axis=1
    )                                                        # (128 d, 1024 h*i)
    diag = np.concatenate(diag_cols, axis=1)                 # (128, 1152)
    wtq_s = wt["q"] * SCALE                                  # (128 i, 1024 d)
    biasp = np.stack(
        [tprime["q"], tprime["k"], tprime["v"], gb, f("out_b")], axis=1
    )                                                        # (128, 5)
    return {
        "wtk": _bf(wt["k"]),
        "wtv": _bf(wt["v"]),
        "gqt": _bf(gqt),
        "outwt": _bf(outwt),
        "wvdm": _bf(wvdm),
        "diag": _bf(diag),
        "biasp": np.ascontiguousarray(biasp),
        "wtqr": _bf(wtq_s),
    }


def _build():
    nc = bacc.Bacc("TRN2", target_bir_lowering=False, debug=False, num_devices=B)
    xb_d = nc.dram_tensor("xb", [128, X + 2], BF16, kind="ExternalInput").ap()
    wtk_d = nc.dram_tensor("wtk", [128, INNER], BF16, kind="ExternalInput").ap()
    wtv_d = nc.dram_tensor("wtv", [128, INNER], BF16, kind="ExternalInput").ap()
    gqt_d = nc.dram_tensor("gqt", [128, INNER], BF16, kind="ExternalInput").ap()
    outwt_d = nc.dram_tensor("outwt", [128, INNER], BF16, kind="ExternalInput").ap()
    wvdm_d = nc.dram_tensor("wvdm", [128, INNER], BF16, kind="ExternalInput").ap()
    diag_d = nc.dram_tensor("diag", [128, 9 * 128], BF16, kind="ExternalInput").ap()
    biasp_d = nc.dram_tensor("biasp", [128, 5], F32, kind="ExternalInput").ap()
    wtqr_d = nc.dram_tensor("wtqr", [128, INNER], BF16, kind="ExternalInput").ap()
    out_d = nc.dram_tensor("out", [128, X], F32, kind="ExternalOutput").ap()

    with tile.TileContext(nc) as tc:
        with (
            tc.tile_pool(name="const", bufs=1) as cp,
            tc.tile_pool(name="gp2", bufs=1) as gp2,
            tc.tile_pool(name="gatep", bufs=1) as gatep,
            tc.tile_pool(name="small", bufs=2) as sp,
        ):
            wtk = cp.tile([128, INNER], BF16)
            wtv = cp.tile([128, INNER], BF16)
            gqt = cp.tile([128, INNER], BF16)
            outwt = cp.tile([128, INNER], BF16)
            wvdm = cp.tile([128, INNER], BF16)
            biasp = cp.tile([128, 5], F32)
            wtqr = cp.tile([128, INNER], BF16)
            ident = cp.tile([128, 128], BF16)
            yq = cp.tile([128, X], BF16, tag="yq")
            yv = cp.tile([128, X], BF16, tag="yv")
            yqt = cp.tile([128, X], BF16, tag="yqt")
            zt = cp.tile([128, 2 * NT * GH], BF16, tag="zt")
            zs = cp.tile([128, 2 * NT], F32, tag="zs")
            z4a = cp.tile([128, 8], F32, tag="z4a")
            z4b = cp.tile([128, 8], F32, tag="z4b")
            zi = cp.tile([128, 2 * NT * GH], F32, tag="zi")
            w3t_sb = cp.tile([128, 128], BF16, tag="w3t")
            sks = [gp2.tile([128, NT * 512], BF16, tag=f"sksb{g}",
                            name=f"sksb{g}") for g in range(GROUPS)]
            gates = [gatep.tile([128, HEADS * 512], BF16, tag=f"gate{c}",
                                name=f"gate{c}") for c in range(NCH)]
            r_sbs = [sp.tile([128, 512], BF16, tag=f"rsb{g}",
                             name=f"rsb{g}") for g in range(GROUPS)]

            make_identity(nc, ident)

            ys = {"q": yq, "v": yv}

            # ---- region 1: conv (q,k,v) + K logits + exp + Z ----
            with (
                tc.tile_pool(name="xp", bufs=1) as xp,
                tc.tile_pool(name="yps", bufs=2, space="PSUM") as yps,
                tc.tile_pool(name="tps", bufs=2, space="PSUM") as tps,
                tc.tile_pool(name="kqps", bufs=3, space="PSUM") as kqps,
            ):
                diag = xp.tile([128, 9 * 128], BF16)
                yk = xp.tile([128, X], BF16, tag="yk")
                xpc = [xp.tile([128, 514], BF16, tag=f"xp{c}", name=f"xpc{c}")
                       for c in range(NCH)]
                nc.sync.dma_start(out=diag, in_=diag_d)
                nc.sync.dma_start(out=biasp, in_=biasp_d)
                for c in range(NCH):
                    nc.sync.dma_start(
                        out=xpc[c], in_=xb_d[:, c * 512:c * 512 + 514])
                for sb_t, dr in ((wtk, wtk_d), (wtqr, wtqr_d), (wvdm, wvdm_d),
                                 (outwt, outwt_d), (gqt, gqt_d), (wtv, wtv_d)):
                    nc.sync.dma_start(out=sb_t, in_=dr)

                for pi, p in enumerate(("q", "k", "v")):
                    for c in range(NCH):
                        pt = yps.tile([128, 512], F32, tag="yps")
                        for j in range(3):
                            dsl = diag[:, (pi * 3 + j) * 128:(pi * 3 + j + 1) * 128]
                            nc.tensor.matmul(
                                pt, dsl, xpc[c][:, j:j + 512],
                                start=(j == 0), stop=(j == 2),
                            )
                        out_t = yk if p == "k" else ys[p]
                        nc.scalar.activation(
                            out_t[:, c * 512:(c + 1) * 512], pt,
                            AF.Identity, bias=biasp[:, pi:pi + 1],
                        )
                        if p == "q":
                            for tt in range(4):
                                t = c * 4 + tt
                                qt = tps.tile([128, 128], BF16, tag="yqt")
                                nc.tensor.transpose(
                                    qt, yq[:, t * 128:(t + 1) * 128], ident,
                                )
                                nc.vector.tensor_copy(
                                    yqt[:, t * 128:(t + 1) * 128], qt
                                )
                def normalize(g, act_mod, t0=0, t1=NT):
                    # normalize sk in place; every act_mod-th slice on Act
                    sksb = sks[g]
                    for t in range(t0, t1):
                        for hh in range(GH):
                            col = g * NT * GH + t * GH + hh
                            sl = sksb[:, t * 512 + hh * 128:
                                      t * 512 + (hh + 1) * 128]
                            if (t * GH + hh) % 16 < act_mod:
                                nc.scalar.activation(
                                    sl, sl, AF.Copy, scale=zi[:, col:col + 1]
                                )
                            else:
                                nc.vector.tensor_scalar(
                                    sl, sl, zi[:, col:col + 1], None, ALU.mult
                                )

                for g in range(GROUPS):
                    osl = slice(g * 512, (g + 1) * 512)
                    for t in range(NT):
                        kps = kqps.tile([128, 512], F32, tag="kq")
                        nc.tensor.matmul(
                            kps, yk[:, t * 128:(t + 1) * 128], wtk[:, osl],
                            start=True, stop=True,
                        )
                        nc.scalar.activation(
                            sks[g][:, t * 512:(t + 1) * 512], kps, AF.Exp,
                        )
                        if t % 4 == 3:
                            t0 = t - 3
                            inr = sks[g][:, t0 * 512:(t + 1) * 512].rearrange(
                                "p (t h d) -> p t h d", t=4, h=GH
                            )
                            zb4 = g * NT * GH + t0 * GH
                            with nc.allow_low_precision("bf16 softmax denom"):
                                nc.vector.tensor_reduce(
                                    zt[:, zb4:zb4 + 4 * GH], inr,
                                    mybir.AxisListType.X, ALU.add,
                                )
                        if t % 8 == 7:
                            zb8 = g * NT * GH + (t - 7) * GH
                            nc.vector.reciprocal(
                                zi[:, zb8:zb8 + 8 * GH], zt[:, zb8:zb8 + 8 * GH]
                            )
                    if g == 0:
                        # g0 normalize rides under g1's EXP window (all-DVE)
                        normalize(0, 0)

            # ---- region 2: normalize + R, overlapped with gate-path ----
            with (
                tc.tile_pool(name="sigp", bufs=2) as sigp,
                tc.tile_pool(name="rps", bufs=1, space="PSUM") as rps,
                tc.tile_pool(name="goutps", bufs=2, space="PSUM") as goutps,
                tc.tile_pool(name="vps", bufs=2, space="PSUM") as vps,
            ):
                def gate_chunk(c):
                    csl = slice(c * 512, (c + 1) * 512)
                    for hp in range(HEADS // 2):
                        v_ps = vps.tile([128, 1024], F32, tag="vp")
                        sig = sigp.tile([128, 1024], BF16, tag="sig")
                        for d in range(2):
                            h = hp * 2 + d
                            g_ps = goutps.tile([128, 512], F32, tag="gout")
                            nc.tensor.matmul(
                                g_ps, gqt[:, h * 128:(h + 1) * 128],
                                yq[:, csl], start=True, stop=True,
                            )
                            nc.scalar.activation(
                                sig[:, d * 512:(d + 1) * 512], g_ps,
                                AF.Sigmoid, bias=biasp[:, 3:4],
                            )
                            nc.tensor.matmul(
                                v_ps[:, d * 512:(d + 1) * 512],
                                wtv[:, h * 128:(h + 1) * 128],
                                yv[:, csl], start=True, stop=True,
                            )
                        nc.vector.tensor_tensor(
                            gates[c][:, hp * 1024:(hp + 1) * 1024], v_ps, sig,
                            ALU.mult,
                        )

                def r_group(g):
                    r_ps = rps.tile([128, 512], F32, tag="r")
                    for t in range(NT):
                        nc.tensor.matmul(
                            r_ps, yqt[:, t * 128:(t + 1) * 128],
                            sks[g][:, t * 512:(t + 1) * 512],
                            start=(t == 0), stop=(t == NT - 1),
                            skip_group_check=True,
                        )
                    nc.vector.tensor_copy(r_sbs[g], r_ps)

                # g0 already normalized during region 1
                gate_chunk(0)
                r_group(0)
                gate_chunk(1)
                normalize(1, 5, 0, NT // 2)
                gate_chunk(2)
                normalize(1, 5, NT // 2, NT)
                r_group(1)
                gate_chunk(3)

            # ---- region 3a: gate chunks overlapped with kat/W3 chain ----
            with (
                tc.tile_pool(name="katps", bufs=1, space="PSUM") as katps,
                tc.tile_pool(name="m2ps", bufs=1, space="PSUM") as m2ps,
                tc.tile_pool(name="w3ps", bufs=1, space="PSUM") as w3ps,
                tc.tile_pool(name="sigp2", bufs=2) as sigp,
                tc.tile_pool(name="goutps2", bufs=2, space="PSUM") as goutps,
                tc.tile_pool(name="vps2", bufs=1, space="PSUM") as vps,
            ):
                def gate_chunk3(c):
                    csl = slice(c * 512, (c + 1) * 512)
                    for hp in range(HEADS // 2):
                        v_ps = vps.tile([128, 1024], F32, tag="vp")
                        sig = sigp.tile([128, 1024], BF16, tag="sig")
                        for d in range(2):
                            h = hp * 2 + d
                            g_ps = goutps.tile([128, 512], F32, tag="gout")
                            nc.tensor.matmul(
                                g_ps, gqt[:, h * 128:(h + 1) * 128],
                                yq[:, csl], start=True, stop=True,
                            )
                            nc.scalar.activation(
                                sig[:, d * 512:(d + 1) * 512], g_ps,
                                AF.Sigmoid, bias=biasp[:, 3:4],
                            )
                            nc.tensor.matmul(
                                v_ps[:, d * 512:(d + 1) * 512],
                                wtv[:, h * 128:(h + 1) * 128],
                                yv[:, csl], start=True, stop=True,
                            )
                        nc.vector.tensor_tensor(
                            gates[c][:, hp * 1024:(hp + 1) * 1024], v_ps, sig,
                            ALU.mult,
                        )

                def kat_chain(g):
                    for hh in range(GH):
                        h = g * GH + hh
                        kat_ps = katps.tile([128, 128], F32, tag="katp")
                        nc.tensor.matmul(
                            kat_ps, wtqr[:, h * 128:(h + 1) * 128],
                            r_sbs[g][:, hh * 128:(hh + 1) * 128],
                            start=True, stop=True, skip_group_check=True,
                        )
                        kat_sb = sp.tile([128, 128], BF16, tag="katsb")
                        nc.vector.tensor_copy(kat_sb, kat_ps)
                        m2_ps = m2ps.tile([128, 128], F32, tag="m2")
                        nc.tensor.matmul(
                            m2_ps, kat_sb,
                            wvdm[:, h * 128:(h + 1) * 128],
                            start=True, stop=True, skip_group_check=True,
                        )
                        m2_sb = sp.tile([128, 128], BF16, tag="m2sb")
                        nc.vector.tensor_copy(m2_sb, m2_ps)
                        nc.tensor.matmul(
                            w3t_ps, m2_sb,
                            outwt[:, h * 128:(h + 1) * 128],
                            start=(h == 0), stop=(h == HEADS - 1),
                            skip_group_check=True,
                        )

                w3t_ps = w3ps.tile([128, 128], F32)
                gate_chunk3(4)
                kat_chain(0)
                gate_chunk3(5)
                kat_chain(1)
                gate_chunk3(6)
                nc.vector.tensor_copy(w3t_sb, w3t_ps)

            # ---- region 3b: final projection per 512-chunk ----
            with (
                tc.tile_pool(name="finps", bufs=2, space="PSUM") as finps,
                tc.tile_pool(name="sigp3", bufs=2) as sigp,
                tc.tile_pool(name="goutps3", bufs=2, space="PSUM") as goutps,
                tc.tile_pool(name="vps3", bufs=1, space="PSUM") as vps,
                tc.tile_pool(name="bpool", bufs=2) as bp,
            ):

                def gate_chunk3(c):
                    csl = slice(c * 512, (c + 1) * 512)
                    for hp in range(HEADS // 2):
                        v_ps = vps.tile([128, 1024], F32, tag="vp")
                        sig = sigp.tile([128, 1024], BF16, tag="sig")
                        for d in range(2):
                            h = hp * 2 + d
                            g_ps = goutps.tile([128, 512], F32, tag="gout")
                            nc.tensor.matmul(
                                g_ps, gqt[:, h * 128:(h + 1) * 128],
                                yq[:, csl], start=True, stop=True,
                            )
                            nc.scalar.activation(
                                sig[:, d * 512:(d + 1) * 512], g_ps,
                                AF.Sigmoid, bias=biasp[:, 3:4],
                            )
                            nc.tensor.matmul(
                                v_ps[:, d * 512:(d + 1) * 512],
                                wtv[:, h * 128:(h + 1) * 128],
                                yv[:, csl], start=True, stop=True,
                            )
                        nc.vector.tensor_tensor(
                            gates[c][:, hp * 1024:(hp + 1) * 1024], v_ps, sig,
                            ALU.mult,
                        )

                def fin_chunk(c):
                    csl = slice(c * 512, (c + 1) * 512)
                    fin_ps = finps.tile([128, 512], F32, tag="fin")
                    nc.tensor.matmul(
                        fin_ps, w3t_sb, yv[:, csl],
                        start=True, stop=False, skip_group_check=True,
                    )
                    for h in range(HEADS):
                        nc.tensor.matmul(
                            fin_ps, outwt[:, h * 128:(h + 1) * 128],
                            gates[c][:, h * 512:(h + 1) * 512],
                            start=False, stop=(h == HEADS - 1),
                            skip_group_check=True,
                        )
                    fin_sb = bp.tile([128, 512], F32, tag="finsb")
                    nc.scalar.activation(
                        fin_sb, fin_ps, AF.Identity, bias=biasp[:, 4:5]
                    )
                    nc.sync.dma_start(out=out_d[:, csl], in_=fin_sb)

                gate_chunk3(7)
                for c in range(NCH):
                    fin_chunk(c)

    nc.compile()
    return nc


def kernel(**inputs):
    global _NC, LAST_EXEC_NS
    host = _prep(inputs)
    if _NC is None:
        _NC = _build()
    x = np.asarray(inputs["x"], np.float32)
    in_maps = []
    for b in range(B):
        xp = np.pad(x[b], ((0, 0), (1, 1)))
        m = {"xb": _bf(xp)}
        m.update(host)
        in_maps.append(m)
    res = run_bass_kernel_spmd(
        _NC, in_maps, core_ids=list(range(B)), trace=TRACE
    )
    LAST_EXEC_NS = res.exec_time_ns
    return np.stack([r["out"] for r in res.results]).astype(np.float32)
